# revision 57
# baseline (speedup 1.0000x reference)
"""Trainium2 Bass kernel for nn_Attention (llama-style attention block, GQA, RoPE).

bf16 dataflow (rel-err gate 2e-2; this kernel lands ~1.05e-2):
  - All matmul operands bf16 (PSUM accumulation stays f32).
  - Attention packs TWO q-heads per 512-wide moving operand (2 x 256
    tokens); causality handled at 256-token granularity.
  - Softmax: bf16 DVE accumulator sums the exp tiles (halves folded by a
    DVE add), ONE ones-matmul per q-block reduces it across partitions;
    the reciprocal is partition-broadcast via a K=1 ones matmul into PSUM
    (no gpsimd op), and the attention output is normalized on the sender.
  - v_nat (tokens-on-partitions V) comes straight from DRAM via the DMA
    transpose xbar: no PE transposes, no PSUM bank, no scalar drains.
  - The attention PE stream is software-pipelined across q-block
    boundaries (pending av pair + two-stage deferred normalization chain)
    so the PE never waits on the DVE chain at block ends.

Queue/collective discipline (the hard-won part):
  - ONE AllToAll per batch.  A collective trigger blocks its issue queue
    until the PREVIOUS collective completes (NRT straight-line order), so
    back-to-back collectives freeze the gpsimd queue when this core runs
    ahead of its peers (launch skew is tens of us).  Per-batch collectives
    are ~100+us apart and never block.
  - A non-Shared AllToAll output gets completion-gated staging copies on
    the TRIGGER'S queue (gpsimd) at the call site: nothing latency-
    sensitive may be emitted on gpsimd between a collective call and the
    point where its data is consumed anyway.  Attention therefore uses NO
    gpsimd ops at all; payload writes go on sync, weight loads on scalar
    (dep-free DMAs can never block a queue head), and the collective-gated
    a_sb loads are the only gpsimd DMAs (batched 8-per-batch -- a large
    pile of pending gated DMAs starves global DMA issue resources).
  - Concurrent DMA traffic directly slows PE matmuls (measured 220 ->
    408 ns/MM under saturating DMA), so wo weights stream only ~1.5x
    (first SPLIT m-blocks run per-batch-half for collective runway, the
    rest run both halves off a single weight load).

Distribution (8 NeuronCores, Megatron-style tensor parallel over heads):
  - Each core: 4 Q heads + matching KV head (wq/wk/wv output-dim sharded).
    Attention per-core in transposed dataflow (head_dim on partitions).
  - Per-batch AllToAlls reshard the attention output token-parallel; each
    core runs wo for its 2x256-token block against the full wo.
"""

import sys

if "/opt/trn_rl_repo" not in sys.path:
    sys.path.insert(0, "/opt/trn_rl_repo")

import numpy as np
import ml_dtypes

BF16 = ml_dtypes.bfloat16

N_CORES = 8
B, S, D = 2, 2048, 4096
N_HEADS = 32
N_KV_HEADS = 8
HEAD_DIM = 128
H_PER_CORE = N_HEADS // N_CORES          # 4 q heads per core
TOK = B * S                              # 4096 flattened tokens
QKV_M = H_PER_CORE * HEAD_DIM + 2 * HEAD_DIM  # 768 projection rows per core
PROJ_TOK = 512                           # token block in the projection stage
SQ_BLK = 512                             # moving width in attention (2 heads x 256)
A2A_TOK = 256                            # tokens per rank per per-batch AllToAll
NJJ = S // A2A_TOK                       # 8 q-blocks of 256 per batch
N_TCHUNK = S // HEAD_DIM                 # 16 key chunks per batch
SCALE = 1.0 / float(np.sqrt(HEAD_DIM))
NKC = D // 128                           # 32 contraction chunks

# partition permutation for RoPE: pair (even, odd) lives 16 partitions apart
# inside a 32-partition quadrant, so the rotation is a single stream_shuffle.
_P = np.arange(128)
_I_OF_P = 16 * (_P // 32) + (_P % 32) % 16          # rope pair index 0..63
_IS_ODD = (_P % 32) >= 16
PERM = (2 * _I_OF_P + _IS_ODD.astype(np.int64)).astype(np.int64)  # orig row in head block
SHUF_MASK = [(i + 16) % 32 for i in range(32)]

_PROGRAMS = {}


def _build_program(mask_mode):
    """Build + compile the SPMD program. mask_mode in {'causal', 'none', 'general'}."""
    import concourse.bass as bass
    import concourse.mybir as mybir
    import concourse.tile as tile
    from concourse import bacc
    from concourse.masks import make_identity

    f32 = mybir.dt.float32
    bf16 = mybir.dt.bfloat16
    Exp = mybir.ActivationFunctionType.Exp

    nc = bacc.Bacc("TRN2", target_bir_lowering=False, debug=False,
                   num_devices=N_CORES)

    xT = nc.dram_tensor("xT", [D, TOK], bf16, kind="ExternalInput")
    wqkvT = nc.dram_tensor("wqkvT", [D, QKV_M], bf16, kind="ExternalInput")
    # wo pre-tiled on host: [m_chunk, p, k_chunk, m_col] so each stationary
    # column-block DMA reads contiguous lines
    woT4 = nc.dram_tensor("woT4", [NKC, 128, NKC, 128], bf16, kind="ExternalInput")
    cos2 = nc.dram_tensor("cos2", [128, S], bf16, kind="ExternalInput")
    sin2 = nc.dram_tensor("sin2", [128, S], bf16, kind="ExternalInput")
    if mask_mode == "general":
        # additive mask stored transposed: maskT[k_pos, q_pos]
        maskT = nc.dram_tensor("maskT", [S, S], f32, kind="ExternalInput")
    out_d = nc.dram_tensor("out", [D, SQ_BLK], bf16, kind="ExternalOutput")

    xT_t = xT.ap().rearrange("(k p) t -> p k t", p=128)      # [128, 32, TOK]
    wqkvT_t = wqkvT.ap().rearrange("(k p) m -> p k m", p=128)  # [128, 32, 768]

    with tile.TileContext(nc) as tc:
        # at_kt/at_kv/at_q are hoisted alongside the projection pools so the
        # attention preamble DMAs (kT, v transposes, q loads) prefetch during
        # the projection stage instead of serializing at the pool boundary.
        with tc.tile_pool(name="const", bufs=1) as const, \
             tc.tile_pool(name="dram", bufs=1, space="DRAM") as dram, \
             tc.tile_pool(name="at_kt", bufs=2) as at_kt, \
             tc.tile_pool(name="at_kv", bufs=2) as at_kv, \
             tc.tile_pool(name="at_q", bufs=4) as at_q:
            # per-core q/k/v (transposed layout), split per batch
            qd = [dram.tile([H_PER_CORE * 128, S], bf16, name=f"qd{b_}")
                  for b_ in range(B)]
            kd = [dram.tile([128, S], bf16, name=f"kd{b_}") for b_ in range(B)]
            vd = [dram.tile([128, S], bf16, name=f"vd{b_}") for b_ in range(B)]
            # ONE payload tile per batch (all 4 heads).  Two collectives, not
            # four: a collective trigger instruction blocks its issue queue
            # until the PREVIOUS collective completes (NRT straight-line
            # ordering), so back-to-back collectives freeze whatever queue
            # hosts the trigger whenever this core runs ahead of its peers.
            # With one collective per batch the triggers are ~100+us apart
            # and never block.  NOTE also: a non-Shared AllToAll output makes
            # the framework emit post-collective staging copies on the SYNC
            # dma queue at the call site, gated on collective completion --
            # any later sync-queue DMA waits behind them.  So every
            # latency-sensitive DMA emitted after the first collective
            # (payload writes, weight loads, a_sb loads) is issued from a
            # NON-sync engine queue instead.
            a2a_in = [dram.tile([N_CORES, 2 * 256, A2A_TOK], bf16,
                                name=f"a2a_in{b_}") for b_ in range(B)]
            a2a_out = [dram.tile([N_CORES, 2 * 256, A2A_TOK], bf16,
                                 name=f"a2a_out{b_}") for b_ in range(B)]

            ones_col = const.tile([128, 1], bf16)     # lhsT for column sums
            nc.vector.memset(ones_col[:], 1.0)
            ones_row = const.tile([1, 128], bf16)     # lhsT for partition bcast
            nc.vector.memset(ones_row[:], 1.0)
            if mask_mode == "causal":
                # multiplicative 0/1 mask for the diagonal chunk-group:
                # cm[p, a, hh, t] = (t - p >= 128*a), same for both packed
                # heads hh
                cm = const.tile([128, 2, 2, A2A_TOK], bf16, name="cm")
                nc.gpsimd.memset(cm[:], 1.0)
                for a in range(2):
                    for hh in range(2):
                        nc.gpsimd.affine_select(
                            out=cm[:, a, hh, :],
                            in_=cm[:, a, hh, :],
                            pattern=[[1, A2A_TOK]], base=-128 * a,
                            channel_multiplier=-1,
                            compare_op=mybir.AluOpType.is_ge, fill=0.0,
                        )

            def emit_preamble(b):
                """kT / v_nat / packed-q loads for one batch's attention.
                Called mid-projection for batch 0 so the DMAs overlap the
                remaining projection blocks.  v_nat (tokens on partitions)
                comes straight from vd via the DMA transpose xbar -- no PE
                transposes, no PSUM bank, no scalar drains."""
                kT = at_kt.tile([128, S], bf16, tag="kT", name=f"kT{b}")
                nc.sync.dma_start(kT[:], kd[b][:])
                v_nat = at_kv.tile([128, N_TCHUNK, 128], bf16,
                                   tag="vn", name=f"vn{b}")
                for i in range(N_TCHUNK):
                    nc.sync.dma_start_transpose(
                        v_nat[:, i, :], vd[b][:, i * 128:(i + 1) * 128])
                qTps = []
                for hp in range(2):
                    qTp = at_q.tile([128, NJJ, 2, A2A_TOK], bf16, tag="qTp",
                                    name=f"qTp{b}_{hp}")
                    for hh in range(2):
                        nc.sync.dma_start(
                            qTp[:, :, hh, :],
                            qd[b][(2 * hp + hh) * 128:
                                  (2 * hp + hh + 1) * 128, :]
                            .rearrange("p (jj t) -> p jj t", jj=NJJ))
                    qTps.append(qTp)
                return kT, v_nat, qTps

            pre = {}

            # ---------------- stage 1: fused QKV projection + RoPE ----------------
            # k-outer / m-inner with 6 live PSUM accumulation groups, so the
            # two half-K x tiles (xA, xB) double-buffer against each other.
            n_blk = TOK // PROJ_TOK   # 8
            HK = NKC // 2
            with tc.tile_pool(name="pj_w", bufs=1) as pj_w, \
                 tc.tile_pool(name="pj_x", bufs=3) as pj_x, \
                 tc.tile_pool(name="pj_cs", bufs=2) as pj_cs, \
                 tc.tile_pool(name="pj_t", bufs=2) as pj_t, \
                 tc.tile_pool(name="pj_o", bufs=2) as pj_o, \
                 tc.tile_pool(name="pj_ps", bufs=8, space="PSUM") as pj_ps:
                w_sb = pj_w.tile([128, NKC, QKV_M], bf16)
                x0 = slice(0, PROJ_TOK)
                xA0 = pj_x.tile([128, HK, PROJ_TOK], bf16, tag="xA")
                xB0 = pj_x.tile([128, HK, PROJ_TOK], bf16, tag="xB", bufs=2)
                # n=0 runs k-outer (below), so interleave x and w loads in
                # matching k-major pieces: the first matmul needs only
                # w[:, 0:2, :] + xA0[:, 0:2, :] (~0.6MB), not the full 10MB
                for kw in range(0, HK, 2):
                    nc.sync.dma_start(xA0[:, kw:kw + 2, :],
                                      xT_t[:, kw:kw + 2, x0])
                    nc.sync.dma_start(w_sb[:, kw:kw + 2, :],
                                      wqkvT_t[:, kw:kw + 2, :])
                for kw in range(HK, NKC, 2):
                    nc.sync.dma_start(xB0[:, kw - HK:kw - HK + 2, :],
                                      xT_t[:, kw:kw + 2, x0])
                    nc.sync.dma_start(w_sb[:, kw:kw + 2, :],
                                      wqkvT_t[:, kw:kw + 2, :])
                for n in range(n_blk):
                    s0 = (n * PROJ_TOK) % S  # position within the batch
                    bn = n // (S // PROJ_TOK)  # batch of this token block
                    cols = slice(n * PROJ_TOK, (n + 1) * PROJ_TOK)
                    bcols = slice(s0, s0 + PROJ_TOK)
                    if n == 0:
                        xA, xB = xA0, xB0
                    else:
                        xA = pj_x.tile([128, HK, PROJ_TOK], bf16, tag="xA")
                        xB = pj_x.tile([128, HK, PROJ_TOK], bf16, tag="xB", bufs=2)
                        nc.sync.dma_start(xA[:], xT_t[:, 0:HK, cols])
                        nc.sync.dma_start(xB[:], xT_t[:, HK:NKC, cols])
                    c_sb = pj_cs.tile([128, PROJ_TOK], bf16, tag="c")
                    s_sb = pj_cs.tile([128, PROJ_TOK], bf16, tag="s")
                    nc.sync.dma_start(c_sb[:], cos2.ap()[:, s0:s0 + PROJ_TOK])
                    nc.sync.dma_start(s_sb[:], sin2.ap()[:, s0:s0 + PROJ_TOK])
                    # n=0: k-outer / m-inner so the first matmul only waits on
                    # one small k-major x/w piece (fast startup ramp).
                    # n>=1: m-outer / k-inner so each head's full contraction
                    # finishes first and its RoPE drain overlaps the next
                    # head's matmuls (keeps the block tail short).
                    if n == 0:
                        pss = [pj_ps.tile([128, PROJ_TOK], f32, tag="ps",
                                          name=f"ps_{n}_{m_}")
                               for m_ in range(QKV_M // 128)]
                        for k in range(NKC):
                            xsb = xA if k < HK else xB
                            xi = k if k < HK else k - HK
                            for m_ in range(QKV_M // 128):
                                nc.tensor.matmul(
                                    pss[m_][:],
                                    w_sb[:, k, m_ * 128:(m_ + 1) * 128],
                                    xsb[:, xi, :],
                                    start=(k == 0), stop=(k == NKC - 1))
                    for m in range(QKV_M // 128):  # q0..q3, k, v
                        if n == 0:
                            ps = pss[m]
                        else:
                            ps = pj_ps.tile([128, PROJ_TOK], f32, tag="ps",
                                            name=f"ps_{n}_{m}")
                            for k in range(NKC):
                                xsb = xA if k < HK else xB
                                xi = k if k < HK else k - HK
                                nc.tensor.matmul(
                                    ps[:], w_sb[:, k, m * 128:(m + 1) * 128],
                                    xsb[:, xi, :],
                                    start=(k == 0), stop=(k == NKC - 1))
                        o_sb = pj_o.tile([128, PROJ_TOK], bf16, tag="o")
                        if m < 5:  # rope for q heads + k
                            tmp = pj_t.tile([128, PROJ_TOK], bf16, tag="tmp")
                            rot = pj_t.tile([128, PROJ_TOK], bf16, tag="rot")
                            t1 = pj_t.tile([128, PROJ_TOK], bf16, tag="t1")
                            nc.scalar.copy(tmp[:], ps[:])
                            nc.vector.stream_shuffle(rot[:], tmp[:], SHUF_MASK)
                            nc.vector.tensor_mul(t1[:], tmp[:], c_sb[:])
                            nc.vector.tensor_mul(rot[:], rot[:], s_sb[:])
                            nc.vector.tensor_add(o_sb[:], t1[:], rot[:])
                        else:
                            nc.scalar.copy(o_sb[:], ps[:])
                        if m < 4:
                            dst = qd[bn][m * 128:(m + 1) * 128, bcols]
                        elif m == 4:
                            dst = kd[bn][:, bcols]
                        else:
                            dst = vd[bn][:, bcols]
                        nc.sync.dma_start(dst, o_sb[:])
                    if n == S // PROJ_TOK - 1:
                        # emit batch-0 attention preamble DMAs here so they
                        # queue ahead of the remaining projection stores and
                        # run as soon as batch 0's q/k/v land in DRAM
                        pre[0] = emit_preamble(0)

            # ---------------- stage 2: attention + per-batch AllToAll --------
            # Two q-heads packed side by side in the 512-wide moving operand
            # (2 x 256 tokens); causality handled at 256-token granularity.
            # wo_w is opened before the attention pools so the wo weight
            # prefetch streams during attention / the collectives. The batch-0
            # collective overlaps batch-1 attention; Pool-stream ordering is
            # kept safe by deferring each hp's normalization broadcast to its
            # end (so it is never queued behind a collective it doesn't need).
            with tc.tile_pool(name="wo_w", bufs=5) as wo_w:
                a_sb = wo_w.tile([128, NKC, SQ_BLK], bf16, tag="a", bufs=1)

                def emit_a_loads(bb):
                    # issued from the gpsimd queue (the sync queue would
                    # stall payload writes behind these collective-gated
                    # loads; the scalar queue lets the tile scheduler
                    # interleave them ahead of batch-1's exps).  Batched to 8
                    # issues: a large pile of pending gated DMA instructions
                    # starves the machine of global DMA issue resources when
                    # the collective is slow (high peer skew).
                    for r_ in range(N_CORES):
                        nc.gpsimd.dma_start(
                            a_sb[:, 4 * r_:4 * r_ + 4,
                                 bb * A2A_TOK:(bb + 1) * A2A_TOK],
                            a2a_out[bb][r_].rearrange("(hh p) t -> p hh t",
                                                      p=128))

                # first wo weight tiles prefetch on the sync queue, emitted
                # BEFORE the first collective call site (after it, the sync
                # queue is blocked by collective staging copies)
                w0s = []
                for m_ in range(5):
                    w0 = wo_w.tile([128, NKC, 128], bf16,
                                   tag="w", bufs=8, name=f"w{m_}_0")
                    nc.sync.dma_start(w0[:], woT4.ap()[m_])
                    w0s.append(w0)
                with tc.tile_pool(name="at_e", bufs=7) as at_e, \
                     tc.tile_pool(name="at_acc", bufs=3) as at_acc, \
                     tc.tile_pool(name="at_nrm", bufs=4) as at_nrm, \
                     tc.tile_pool(name="at_o", bufs=3) as at_o, \
                     tc.tile_pool(name="at_mt", bufs=4) as at_mt, \
                     tc.tile_pool(name="ps_s", bufs=2, space="PSUM") as ps_s, \
                     tc.tile_pool(name="ps_av", bufs=2, space="PSUM") as ps_av, \
                     tc.tile_pool(name="ps_sm", bufs=1, space="PSUM") as ps_sm, \
                     tc.tile_pool(name="ps_rb", bufs=1, space="PSUM") as ps_rb:
                    for b in range(B):
                        if b not in pre:
                            pre[b] = emit_preamble(b)
                        if b + 1 < B and b + 1 not in pre:
                            # emit the next batch's preamble loads now, ahead
                            # of this batch's at-dmas in the DMA queue, so a
                            # stalled normalization chain can't delay them
                            pre[b + 1] = emit_preamble(b + 1)
                        kT, v_nat, qTps = pre[b]
                        for hp in range(2):
                            qTp = qTps[hp]

                            # pending av pair, carried across q-block (jj)
                            # boundaries so the PE stream never drains:
                            # (av_tile, c0, e_tile, is_last, accT, jj)
                            prev = [None]
                            # two-deep deferred normalization chain.  Stage 1
                            # (post): avr copy + denominator reduce + recip.
                            # Stage 2 (post2), one group later: the
                            # partition-broadcast matmul + final scale + DMA,
                            # so the broadcast matmul never waits on the DVE
                            # recip at the PE queue head.  No gpsimd ops at
                            # all: the gpsimd queue carries collective
                            # triggers + their completion-gated staging
                            # copies, which would stall anything behind them.
                            post = [None]
                            post2 = [None]

                            def emit_post2(b=b, hp=hp):
                                if post2[0] is None:
                                    return
                                avr, smb, pjj = post2[0]
                                rbp = ps_rb.tile([128, SQ_BLK], f32,
                                                 tag="rbp", name="rbp")
                                nc.tensor.matmul(rbp[:], ones_row[:], smb[:],
                                                 start=True, stop=True)
                                at = at_o.tile([128, SQ_BLK], bf16,
                                               tag="at", name="at")
                                nc.vector.tensor_mul(at[:], avr[:], rbp[:])
                                nc.sync.dma_start(
                                    a2a_in[b][pjj,
                                              hp * 256:(hp + 1) * 256, :]
                                    .rearrange("(hh p) t -> p hh t", p=128),
                                    at[:].rearrange("p (hh t) -> p hh t",
                                                    hh=2))
                                post2[0] = None

                            def emit_post(emit_post2=emit_post2):
                                if post[0] is None:
                                    return
                                pav, pacc, pjj = post[0]
                                # the avr copy releases the av PSUM bank
                                avr = at_o.tile([128, SQ_BLK], bf16,
                                                tag="avr", name="avr")
                                nc.vector.tensor_copy(avr[:], pav[:])
                                sm = ps_sm.tile([1, SQ_BLK], f32,
                                                tag="sm", name="sm")
                                nc.tensor.matmul(sm[:], ones_col[:],
                                                 pacc[:, 0, :],
                                                 start=True, stop=True)
                                rs = at_nrm.tile([1, SQ_BLK], f32,
                                                 tag="rs", name="rs")
                                nc.vector.reciprocal_approx_fast(
                                    out=rs[:], in_=sm[:])
                                smb = at_nrm.tile([1, SQ_BLK], bf16,
                                                  tag="smb", name="smb")
                                nc.vector.tensor_copy(smb[:], rs[:])
                                emit_post2()
                                post2[0] = (avr, smb, pjj)
                                post[0] = None

                            def emit_av(nxt=None, v_nat=v_nat):
                                if prev[0] is None:
                                    prev[0] = nxt
                                    return
                                pav, pc0, pe, plast, pacc, pjj = prev[0]
                                nc.tensor.matmul(
                                    pav[:], v_nat[:, pc0, :], pe[:, 0, :],
                                    start=(pc0 == 0), stop=False)
                                nc.tensor.matmul(
                                    pav[:], v_nat[:, pc0 + 1, :], pe[:, 1, :],
                                    start=False, stop=plast)
                                emit_post()
                                if plast:
                                    post[0] = (pav, pacc, pjj)
                                prev[0] = nxt

                            # descending jj: the deep-pipeline blocks come
                            # first (better PE ramp), the shallow ones drain
                            # quickly right before the collective fires
                            for jj in reversed(range(NJJ)):
                                G = (jj + 1 if mask_mode == "causal"
                                     else N_TCHUNK // 2)
                                # (gpsimd elementwise measured 2-3x slower
                                # than DVE -- keep the exp-tile chain on DVE)
                                ee = nc.vector
                                qs = qTp[:, jj, :, :].rearrange(
                                    "p hh t -> p (hh t)")
                                accT = at_acc.tile([128, 2, SQ_BLK], bf16,
                                                   tag="accT")
                                av = ps_av.tile([128, SQ_BLK], f32, tag="av")
                                for g in range(G):
                                    c0 = 2 * g
                                    sp = ps_s.tile([128, 2, SQ_BLK], f32,
                                                   tag="s")
                                    nc.tensor.matmul(
                                        sp[:, 0, :],
                                        kT[:, c0 * 128:(c0 + 1) * 128],
                                        qs, start=True, stop=True)
                                    nc.tensor.matmul(
                                        sp[:, 1, :],
                                        kT[:, (c0 + 1) * 128:(c0 + 2) * 128],
                                        qs, start=True, stop=True)
                                    sp_flat = sp[:].rearrange(
                                        "p a q -> p (a q)")
                                    if mask_mode == "general":
                                        mt = at_mt.tile([128, 2, 2, A2A_TOK],
                                                        f32, tag="mt")
                                        for hh in range(2):
                                            nc.sync.dma_start(
                                                mt[:, :, hh, :],
                                                maskT.ap()[
                                                    c0 * 128:(c0 + 2) * 128,
                                                    jj * A2A_TOK:
                                                    (jj + 1) * A2A_TOK]
                                                .rearrange("(c p) q -> p c q",
                                                           p=128))
                                        nc.vector.tensor_add(
                                            sp_flat, sp_flat,
                                            mt[:].rearrange(
                                                "p c hh q -> p (c hh q)"))
                                    # first group's exp writes the
                                    # accumulator directly (except G==1,
                                    # where accT[:, 0, :] must become the
                                    # fold -- use a separate e there)
                                    e = (accT if g == 0 and G > 1 else
                                         at_e.tile([128, 2, SQ_BLK], bf16,
                                                   tag="e"))
                                    e_flat = e[:].rearrange("p a q -> p (a q)")
                                    nc.scalar.activation(e_flat, sp_flat, Exp,
                                                         scale=SCALE)
                                    if mask_mode == "causal" and g == G - 1:
                                        ee.tensor_mul(
                                            e_flat, e_flat,
                                            cm[:].rearrange(
                                                "p a hh t -> p (a hh t)"))
                                    # flush the pending av pair BEFORE the
                                    # accumulator update: when g==1 the
                                    # pending e aliases accT and must be
                                    # consumed before accT += e
                                    emit_av((av, c0, e, g == G - 1,
                                             accT, jj))
                                    if g > 0:
                                        ee.tensor_add(accT[:], accT[:],
                                                      e[:])
                                # fold the accumulator halves right after the
                                # last add, so the deferred (single) sm
                                # matmul reads a long-ready input
                                if G > 1:
                                    nc.vector.tensor_add(accT[:, 0, :],
                                                         accT[:, 0, :],
                                                         accT[:, 1, :])
                                else:
                                    nc.vector.tensor_add(accT[:, 0, :],
                                                         e[:, 0, :],
                                                         e[:, 1, :])
                            emit_av()
                            emit_post()
                            emit_post2()
                        # one collective per batch: triggers are ~100+us
                        # apart, so a trigger never waits on the previous
                        # collective (which would block this queue)
                        nc.gpsimd.collective_compute(
                            "AllToAll", mybir.AluOpType.bypass,
                            replica_groups=[list(range(N_CORES))],
                            ins=[a2a_in[b].opt()],
                            outs=[a2a_out[b].opt()],
                        )
                        # batch-0 a_sb loads directly behind the staging
                        # copies on the gpsimd queue -- exactly their data
                        # dependency.  (Batch-1's are deferred to between the
                        # wo passes: 32 pending gated DMA issues starve the
                        # weight-load stream of issue resources.)
                        if b == 0:
                            emit_a_loads(b)


                # ------------- stage 3: wo projection (2x256 owned tokens) ----
                # The first SPLIT m-blocks run on batch-0 tokens only (the
                # first collective's output) to cover the second collective's
                # latency; the rest run full-width with a single weight load;
                # then the split blocks' batch-1 halves re-stream their
                # weights.  Keeps ~50us of collective runway while loading
                # only 1.4x the wo weights (DMA traffic contends directly
                # with PE operand reads -- fewer bytes = faster matmuls).
                SPLIT = 16
                with tc.tile_pool(name="wo_o", bufs=4) as wo_o, \
                     tc.tile_pool(name="wo_ps", bufs=4, space="PSUM") as wo_ps:

                    def emit_wo(m, bb, w_sb2):
                        # bb None = both halves (still as per-half 256-wide
                        # matmuls: a 512-wide read spanning two separately
                        # batch-loaded a_sb halves raced intermittently)
                        bbs = [0, 1] if bb is None else [bb]
                        cols = slice(bbs[0] * A2A_TOK,
                                     (bbs[-1] + 1) * A2A_TOK)
                        w = cols.stop - cols.start
                        ps = wo_ps.tile([128, SQ_BLK], f32, tag="ps",
                                        name=f"ps{m}_{bb}")
                        for kc in range(NKC):
                            for bi in bbs:
                                c2 = slice(bi * A2A_TOK, (bi + 1) * A2A_TOK)
                                nc.tensor.matmul(
                                    ps[:, (bi - bbs[0]) * A2A_TOK:
                                       (bi - bbs[0] + 1) * A2A_TOK],
                                    w_sb2[:, kc, :], a_sb[:, kc, c2],
                                    start=(kc == 0 and bi == bbs[0]),
                                    stop=(kc == NKC - 1 and bi == bbs[-1]))
                        o_sb = wo_o.tile([128, SQ_BLK], bf16, tag="o")
                        nc.vector.tensor_copy(o_sb[:, 0:w], ps[:, 0:w])
                        nc.sync.dma_start(
                            out_d.ap()[m * 128:(m + 1) * 128, cols],
                            o_sb[:, 0:w])

                    def load_w(m, bb):
                        # dep-free weight loads on the scalar queue: idle
                        # after attention's exps, and a dep-free DMA can
                        # never block the queue head
                        w_sb2 = wo_w.tile([128, NKC, 128], bf16,
                                          tag="w", bufs=8,
                                          name=f"w{m}_{bb}")
                        nc.scalar.dma_start(w_sb2[:], woT4.ap()[m])
                        return w_sb2

                    for m in range(SPLIT):
                        emit_wo(m, 0, w0s[m] if m < 5 else load_w(m, 0))
                    # batch-1 a_sb loads: deferred past the runway blocks so
                    # their collective-gated issues don't starve the weight
                    # stream of DMA issue resources
                    emit_a_loads(1)
                    for m in range(SPLIT, NKC):
                        emit_wo(m, None, load_w(m, None))
                    for m in range(SPLIT):
                        emit_wo(m, 1, load_w(m, 1))

    nc.compile()
    return nc


def _get_program(mask_mode):
    if mask_mode not in _PROGRAMS:
        _PROGRAMS[mask_mode] = _build_program(mask_mode)
    return _PROGRAMS[mask_mode]


def _classify_mask(m2):
    if not m2.any():
        return "none"
    causal_ref = np.triu(np.full((S, S), -1e9, dtype=np.float32), k=1)
    return "causal" if np.array_equal(m2, causal_ref) else "general"


def _prep_inputs(x, freqs_cos, freqs_sin, mask, wq, wk, wv, wo):
    """Host-side sharding / layout prep shared by kernel() and test.py."""
    m2 = np.asarray(mask, np.float32).reshape(S, S)
    mask_mode = _classify_mask(m2)

    xT = np.ascontiguousarray(
        np.asarray(x, np.float32).reshape(TOK, D).T).astype(BF16)
    woT = np.asarray(wo, np.float32).T          # [hd_in, D_out]
    # pre-tile wo for contiguous stationary-block DMAs:
    # woT4[m, p, k, mcol] = woT[k*128+p, m*128+mcol]
    woT4 = np.ascontiguousarray(
        woT.reshape(NKC, 128, NKC, 128).transpose(2, 1, 0, 3)).astype(BF16)

    fc = np.asarray(freqs_cos, np.float32)
    fs = np.asarray(freqs_sin, np.float32)
    cos2 = np.ascontiguousarray(fc.T[_I_OF_P, :]).astype(BF16)    # [128, S]
    sgn = np.where(_IS_ODD, 1.0, -1.0).astype(np.float32)[:, None]
    sin2 = np.ascontiguousarray(fs.T[_I_OF_P, :] * sgn).astype(BF16)

    def permute_heads(w):
        w4 = np.asarray(w, np.float32).reshape(-1, HEAD_DIM, D)
        return w4[:, PERM, :].reshape(-1, D)

    wq_p = permute_heads(wq)
    wk_p = permute_heads(wk)
    wv = np.asarray(wv, np.float32)

    in_maps = []
    for c in range(N_CORES):
        wqkvT = np.ascontiguousarray(np.concatenate(
            [wq_p[c * 512:(c + 1) * 512], wk_p[c * 128:(c + 1) * 128],
             wv[c * 128:(c + 1) * 128]], axis=0).T).astype(BF16)   # [D, 768]
        m = {"xT": xT, "wqkvT": wqkvT, "woT4": woT4, "cos2": cos2, "sin2": sin2}
        if mask_mode == "general":
            m["maskT"] = np.ascontiguousarray(m2.T)
        in_maps.append(m)
    return mask_mode, in_maps


def kernel(x, start_pos, freqs_cos, freqs_sin, mask, cache_k, cache_v,
           wq, wk, wv, wo):
    from concourse.bass_utils import run_bass_kernel_spmd

    assert int(start_pos) == 0, "kernel compiled for start_pos == 0"
    mask_mode, in_maps = _prep_inputs(x, freqs_cos, freqs_sin, mask,
                                      wq, wk, wv, wo)
    nc = _get_program(mask_mode)
    res = run_bass_kernel_spmd(nc, in_maps, list(range(N_CORES)))
    out = np.empty((TOK, D), dtype=np.float32)
    for c in range(N_CORES):
        blk = np.asarray(res.results[c]["out"]).astype(np.float32)  # [D, 512]
        for b in range(B):
            rows = slice(b * S + A2A_TOK * c, b * S + A2A_TOK * (c + 1))
            out[rows, :] = blk[:, b * A2A_TOK:(b + 1) * A2A_TOK].T
    return out.reshape(B, S, D)



# revision 61
# speedup vs baseline: 1.0097x; 1.0097x over previous
"""Trainium2 Bass kernel for nn_Attention (llama-style attention block, GQA, RoPE).

bf16 dataflow (rel-err gate 2e-2; this kernel lands ~1.05e-2):
  - All matmul operands bf16 (PSUM accumulation stays f32).
  - Attention packs TWO q-heads per 512-wide moving operand (2 x 256
    tokens); causality handled at 256-token granularity.
  - Softmax: bf16 DVE accumulator sums the exp tiles (halves folded by a
    DVE add), ONE ones-matmul per q-block reduces it across partitions;
    the reciprocal is partition-broadcast via a K=1 ones matmul into PSUM
    (no gpsimd op), and the attention output is normalized on the sender.
  - v_nat (tokens-on-partitions V) comes straight from DRAM via the DMA
    transpose xbar: no PE transposes, no PSUM bank, no scalar drains.
  - The attention PE stream is software-pipelined across q-block
    boundaries (pending av pair + two-stage deferred normalization chain)
    so the PE never waits on the DVE chain at block ends.

Queue/collective discipline (the hard-won part):
  - ONE AllToAll per batch.  A collective trigger blocks its issue queue
    until the PREVIOUS collective completes (NRT straight-line order), so
    back-to-back collectives freeze the gpsimd queue when this core runs
    ahead of its peers (launch skew is tens of us).  Per-batch collectives
    are ~100+us apart and never block.
  - A non-Shared AllToAll output gets completion-gated staging copies on
    the TRIGGER'S queue (gpsimd) at the call site: nothing latency-
    sensitive may be emitted on gpsimd between a collective call and the
    point where its data is consumed anyway.  Attention therefore uses NO
    gpsimd ops at all; payload writes go on sync, weight loads on scalar
    (dep-free DMAs can never block a queue head), and the collective-gated
    a_sb loads are the only gpsimd DMAs (batched 8-per-batch -- a large
    pile of pending gated DMAs starves global DMA issue resources).
  - Concurrent DMA traffic directly slows PE matmuls (measured 220 ->
    408 ns/MM under saturating DMA), so wo weights stream only ~1.5x
    (first SPLIT m-blocks run per-batch-half for collective runway, the
    rest run both halves off a single weight load).

Distribution (8 NeuronCores, Megatron-style tensor parallel over heads):
  - Each core: 4 Q heads + matching KV head (wq/wk/wv output-dim sharded).
    Attention per-core in transposed dataflow (head_dim on partitions).
  - Per-batch AllToAlls reshard the attention output token-parallel; each
    core runs wo for its 2x256-token block against the full wo.
"""

import sys

if "/opt/trn_rl_repo" not in sys.path:
    sys.path.insert(0, "/opt/trn_rl_repo")

import numpy as np
import ml_dtypes

BF16 = ml_dtypes.bfloat16

N_CORES = 8
B, S, D = 2, 2048, 4096
N_HEADS = 32
N_KV_HEADS = 8
HEAD_DIM = 128
H_PER_CORE = N_HEADS // N_CORES          # 4 q heads per core
TOK = B * S                              # 4096 flattened tokens
QKV_M = H_PER_CORE * HEAD_DIM + 2 * HEAD_DIM  # 768 projection rows per core
PROJ_TOK = 512                           # token block in the projection stage
SQ_BLK = 512                             # moving width in attention (2 heads x 256)
A2A_TOK = 256                            # tokens per rank per per-batch AllToAll
NJJ = S // A2A_TOK                       # 8 q-blocks of 256 per batch
N_TCHUNK = S // HEAD_DIM                 # 16 key chunks per batch
SCALE = 1.0 / float(np.sqrt(HEAD_DIM))
NKC = D // 128                           # 32 contraction chunks

# partition permutation for RoPE: pair (even, odd) lives 16 partitions apart
# inside a 32-partition quadrant, so the rotation is a single stream_shuffle.
_P = np.arange(128)
_I_OF_P = 16 * (_P // 32) + (_P % 32) % 16          # rope pair index 0..63
_IS_ODD = (_P % 32) >= 16
PERM = (2 * _I_OF_P + _IS_ODD.astype(np.int64)).astype(np.int64)  # orig row in head block
SHUF_MASK = [(i + 16) % 32 for i in range(32)]

_PROGRAMS = {}


def _build_program(mask_mode):
    """Build + compile the SPMD program. mask_mode in {'causal', 'none', 'general'}."""
    import concourse.bass as bass
    import concourse.mybir as mybir
    import concourse.tile as tile
    from concourse import bacc
    from concourse.masks import make_identity

    f32 = mybir.dt.float32
    bf16 = mybir.dt.bfloat16
    Exp = mybir.ActivationFunctionType.Exp

    nc = bacc.Bacc("TRN2", target_bir_lowering=False, debug=False,
                   num_devices=N_CORES)

    xT = nc.dram_tensor("xT", [D, TOK], bf16, kind="ExternalInput")
    wqkvT = nc.dram_tensor("wqkvT", [D, QKV_M], bf16, kind="ExternalInput")
    # wo pre-tiled on host: [m_chunk, p, k_chunk, m_col] so each stationary
    # column-block DMA reads contiguous lines
    woT4 = nc.dram_tensor("woT4", [NKC, 128, NKC, 128], bf16, kind="ExternalInput")
    cos2 = nc.dram_tensor("cos2", [128, S], bf16, kind="ExternalInput")
    sin2 = nc.dram_tensor("sin2", [128, S], bf16, kind="ExternalInput")
    if mask_mode == "general":
        # additive mask stored transposed: maskT[k_pos, q_pos]
        maskT = nc.dram_tensor("maskT", [S, S], f32, kind="ExternalInput")
    out_d = nc.dram_tensor("out", [D, SQ_BLK], bf16, kind="ExternalOutput")

    xT_t = xT.ap().rearrange("(k p) t -> p k t", p=128)      # [128, 32, TOK]
    wqkvT_t = wqkvT.ap().rearrange("(k p) m -> p k m", p=128)  # [128, 32, 768]

    with tile.TileContext(nc) as tc:
        # at_kt/at_kv/at_q are hoisted alongside the projection pools so the
        # attention preamble DMAs (kT, v transposes, q loads) prefetch during
        # the projection stage instead of serializing at the pool boundary.
        with tc.tile_pool(name="const", bufs=1) as const, \
             tc.tile_pool(name="dram", bufs=1, space="DRAM") as dram, \
             tc.tile_pool(name="at_kt", bufs=2) as at_kt, \
             tc.tile_pool(name="at_kv", bufs=2) as at_kv, \
             tc.tile_pool(name="at_q", bufs=4) as at_q:
            # per-core v (transposed layout), split per batch; q/k stay in
            # SBUF (written directly by the projection RoPE)
            vd = [dram.tile([128, S], bf16, name=f"vd{b_}") for b_ in range(B)]
            # ONE payload tile per batch (all 4 heads).  Two collectives, not
            # four: a collective trigger instruction blocks its issue queue
            # until the PREVIOUS collective completes (NRT straight-line
            # ordering), so back-to-back collectives freeze whatever queue
            # hosts the trigger whenever this core runs ahead of its peers.
            # With one collective per batch the triggers are ~100+us apart
            # and never block.  NOTE also: a non-Shared AllToAll output makes
            # the framework emit post-collective staging copies on the SYNC
            # dma queue at the call site, gated on collective completion --
            # any later sync-queue DMA waits behind them.  So every
            # latency-sensitive DMA emitted after the first collective
            # (payload writes, weight loads, a_sb loads) is issued from a
            # NON-sync engine queue instead.
            a2a_in = [dram.tile([N_CORES, 2 * 256, A2A_TOK], bf16,
                                name=f"a2a_in{b_}") for b_ in range(B)]
            a2a_out = [dram.tile([N_CORES, 2 * 256, A2A_TOK], bf16,
                                 name=f"a2a_out{b_}") for b_ in range(B)]

            ones_col = const.tile([128, 1], bf16)     # lhsT for column sums
            nc.vector.memset(ones_col[:], 1.0)
            ones_row = const.tile([1, 128], bf16)     # lhsT for partition bcast
            nc.vector.memset(ones_row[:], 1.0)
            if mask_mode == "causal":
                # multiplicative 0/1 mask for the diagonal chunk-group:
                # cm[p, a, hh, t] = (t - p >= 128*a), same for both packed
                # heads hh
                cm = const.tile([128, 2, 2, A2A_TOK], bf16, name="cm")
                nc.gpsimd.memset(cm[:], 1.0)
                for a in range(2):
                    for hh in range(2):
                        nc.gpsimd.affine_select(
                            out=cm[:, a, hh, :],
                            in_=cm[:, a, hh, :],
                            pattern=[[1, A2A_TOK]], base=-128 * a,
                            channel_multiplier=-1,
                            compare_op=mybir.AluOpType.is_ge, fill=0.0,
                        )

            # q and k skip the DRAM roundtrip entirely: the projection's
            # RoPE adds write these attention tiles directly (their layouts
            # are clean slices of the projection output), saving 10MB of
            # DMA in the PE-critical projection phase.  Only v round-trips
            # (it needs the DMA transpose xbar, which reads DRAM).
            kT_t = [at_kt.tile([128, S], bf16, tag="kT", name=f"kT{b_}")
                    for b_ in range(B)]
            qT_t = [[at_q.tile([128, NJJ, 2, A2A_TOK], bf16, tag="qTp",
                               name=f"qTp{b_}_{hp_}") for hp_ in range(2)]
                    for b_ in range(B)]

            def emit_preamble(b):
                """v_nat transpose-loads for one batch's attention (tokens
                on partitions, via the DMA transpose xbar -- no PE
                transposes, no PSUM bank, no scalar drains)."""
                v_nat = at_kv.tile([128, N_TCHUNK, 128], bf16,
                                   tag="vn", name=f"vn{b}")
                for i in range(N_TCHUNK):
                    nc.sync.dma_start_transpose(
                        v_nat[:, i, :], vd[b][:, i * 128:(i + 1) * 128])
                return kT_t[b], v_nat, qT_t[b]

            pre = {}

            # ---------------- stage 1: fused QKV projection + RoPE ----------------
            # k-outer / m-inner with 6 live PSUM accumulation groups, so the
            # two half-K x tiles (xA, xB) double-buffer against each other.
            n_blk = TOK // PROJ_TOK   # 8
            HK = NKC // 2
            with tc.tile_pool(name="pj_w", bufs=1) as pj_w, \
                 tc.tile_pool(name="pj_x", bufs=3) as pj_x, \
                 tc.tile_pool(name="pj_cs", bufs=2) as pj_cs, \
                 tc.tile_pool(name="pj_t", bufs=2) as pj_t, \
                 tc.tile_pool(name="pj_o", bufs=2) as pj_o, \
                 tc.tile_pool(name="pj_ps", bufs=8, space="PSUM") as pj_ps:
                w_sb = pj_w.tile([128, NKC, QKV_M], bf16)
                x0 = slice(0, PROJ_TOK)
                xA0 = pj_x.tile([128, HK, PROJ_TOK], bf16, tag="xA")
                xB0 = pj_x.tile([128, HK, PROJ_TOK], bf16, tag="xB", bufs=2)
                # n=0 runs k-outer (below), so interleave x and w loads in
                # matching k-major pieces: the first matmul needs only
                # w[:, 0:2, :] + xA0[:, 0:2, :] (~0.6MB), not the full 10MB
                for kw in range(0, HK, 2):
                    nc.sync.dma_start(xA0[:, kw:kw + 2, :],
                                      xT_t[:, kw:kw + 2, x0])
                    nc.sync.dma_start(w_sb[:, kw:kw + 2, :],
                                      wqkvT_t[:, kw:kw + 2, :])
                for kw in range(HK, NKC, 2):
                    nc.sync.dma_start(xB0[:, kw - HK:kw - HK + 2, :],
                                      xT_t[:, kw:kw + 2, x0])
                    nc.sync.dma_start(w_sb[:, kw:kw + 2, :],
                                      wqkvT_t[:, kw:kw + 2, :])
                for n in range(n_blk):
                    s0 = (n * PROJ_TOK) % S  # position within the batch
                    bn = n // (S // PROJ_TOK)  # batch of this token block
                    cols = slice(n * PROJ_TOK, (n + 1) * PROJ_TOK)
                    bcols = slice(s0, s0 + PROJ_TOK)
                    if n == 0:
                        xA, xB = xA0, xB0
                    else:
                        xA = pj_x.tile([128, HK, PROJ_TOK], bf16, tag="xA")
                        xB = pj_x.tile([128, HK, PROJ_TOK], bf16, tag="xB", bufs=2)
                        nc.sync.dma_start(xA[:], xT_t[:, 0:HK, cols])
                        nc.sync.dma_start(xB[:], xT_t[:, HK:NKC, cols])
                    c_sb = pj_cs.tile([128, PROJ_TOK], bf16, tag="c")
                    s_sb = pj_cs.tile([128, PROJ_TOK], bf16, tag="s")
                    nc.sync.dma_start(c_sb[:], cos2.ap()[:, s0:s0 + PROJ_TOK])
                    nc.sync.dma_start(s_sb[:], sin2.ap()[:, s0:s0 + PROJ_TOK])
                    # n=0: k-outer / m-inner so the first matmul only waits on
                    # one small k-major x/w piece (fast startup ramp).
                    # n>=1: m-outer / k-inner so each head's full contraction
                    # finishes first and its RoPE drain overlaps the next
                    # head's matmuls (keeps the block tail short).
                    if n == 0:
                        pss = [pj_ps.tile([128, PROJ_TOK], f32, tag="ps",
                                          name=f"ps_{n}_{m_}")
                               for m_ in range(QKV_M // 128)]
                        for k in range(NKC):
                            xsb = xA if k < HK else xB
                            xi = k if k < HK else k - HK
                            for m_ in range(QKV_M // 128):
                                nc.tensor.matmul(
                                    pss[m_][:],
                                    w_sb[:, k, m_ * 128:(m_ + 1) * 128],
                                    xsb[:, xi, :],
                                    start=(k == 0), stop=(k == NKC - 1))
                    for m in range(QKV_M // 128):  # q0..q3, k, v
                        if n == 0:
                            ps = pss[m]
                        else:
                            ps = pj_ps.tile([128, PROJ_TOK], f32, tag="ps",
                                            name=f"ps_{n}_{m}")
                            for k in range(NKC):
                                xsb = xA if k < HK else xB
                                xi = k if k < HK else k - HK
                                nc.tensor.matmul(
                                    ps[:], w_sb[:, k, m * 128:(m + 1) * 128],
                                    xsb[:, xi, :],
                                    start=(k == 0), stop=(k == NKC - 1))
                        if m < 5:  # rope for q heads + k
                            # the final RoPE add writes the attention SBUF
                            # tile directly -- no DRAM roundtrip for q/k
                            if m < 4:
                                jj0 = s0 // A2A_TOK
                                dst_sb = (qT_t[bn][m // 2]
                                          [:, jj0:jj0 + 2, m % 2, :])
                                view = lambda a: a.rearrange(
                                    "p (jj t) -> p jj t", jj=2)
                            else:
                                dst_sb = kT_t[bn][:, s0:s0 + PROJ_TOK]
                                view = lambda a: a
                            tmp = pj_t.tile([128, PROJ_TOK], bf16, tag="tmp")
                            rot = pj_t.tile([128, PROJ_TOK], bf16, tag="rot")
                            t1 = pj_t.tile([128, PROJ_TOK], bf16, tag="t1")
                            nc.scalar.copy(tmp[:], ps[:])
                            nc.vector.stream_shuffle(rot[:], tmp[:], SHUF_MASK)
                            nc.vector.tensor_mul(t1[:], tmp[:], c_sb[:])
                            nc.vector.tensor_mul(rot[:], rot[:], s_sb[:])
                            nc.vector.tensor_add(dst_sb, view(t1[:]),
                                                 view(rot[:]))
                        else:
                            o_sb = pj_o.tile([128, PROJ_TOK], bf16, tag="o")
                            nc.scalar.copy(o_sb[:], ps[:])
                            nc.sync.dma_start(vd[bn][:, bcols], o_sb[:])
                    if n == S // PROJ_TOK - 1:
                        # emit batch-0 attention preamble DMAs here so they
                        # queue ahead of the remaining projection stores and
                        # run as soon as batch 0's q/k/v land in DRAM
                        pre[0] = emit_preamble(0)

            # ---------------- stage 2: attention + per-batch AllToAll --------
            # Two q-heads packed side by side in the 512-wide moving operand
            # (2 x 256 tokens); causality handled at 256-token granularity.
            # wo_w is opened before the attention pools so the wo weight
            # prefetch streams during attention / the collectives. The batch-0
            # collective overlaps batch-1 attention; Pool-stream ordering is
            # kept safe by deferring each hp's normalization broadcast to its
            # end (so it is never queued behind a collective it doesn't need).
            with tc.tile_pool(name="wo_w", bufs=5) as wo_w:
                a_sb = wo_w.tile([128, NKC, SQ_BLK], bf16, tag="a", bufs=1)

                def emit_a_loads(bb):
                    # issued from the gpsimd queue (the sync queue would
                    # stall payload writes behind these collective-gated
                    # loads; the scalar queue lets the tile scheduler
                    # interleave them ahead of batch-1's exps).  Batched to 8
                    # issues: a large pile of pending gated DMA instructions
                    # starves the machine of global DMA issue resources when
                    # the collective is slow (high peer skew).
                    for r_ in range(N_CORES):
                        nc.gpsimd.dma_start(
                            a_sb[:, 4 * r_:4 * r_ + 4,
                                 bb * A2A_TOK:(bb + 1) * A2A_TOK],
                            a2a_out[bb][r_].rearrange("(hh p) t -> p hh t",
                                                      p=128))

                # first wo weight tiles prefetch on the sync queue, emitted
                # BEFORE the first collective call site (after it, the sync
                # queue is blocked by collective staging copies)
                w0s = []
                for m_ in range(5):
                    w0 = wo_w.tile([128, NKC, 128], bf16,
                                   tag="w", bufs=8, name=f"w{m_}_0")
                    nc.sync.dma_start(w0[:], woT4.ap()[m_])
                    w0s.append(w0)
                with tc.tile_pool(name="at_e", bufs=7) as at_e, \
                     tc.tile_pool(name="at_acc", bufs=3) as at_acc, \
                     tc.tile_pool(name="at_nrm", bufs=4) as at_nrm, \
                     tc.tile_pool(name="at_o", bufs=3) as at_o, \
                     tc.tile_pool(name="at_mt", bufs=4) as at_mt, \
                     tc.tile_pool(name="ps_s", bufs=2, space="PSUM") as ps_s, \
                     tc.tile_pool(name="ps_av", bufs=2, space="PSUM") as ps_av, \
                     tc.tile_pool(name="ps_sm", bufs=1, space="PSUM") as ps_sm, \
                     tc.tile_pool(name="ps_rb", bufs=1, space="PSUM") as ps_rb:
                    for b in range(B):
                        if b not in pre:
                            pre[b] = emit_preamble(b)
                        if b + 1 < B and b + 1 not in pre:
                            # emit the next batch's preamble loads now, ahead
                            # of this batch's at-dmas in the DMA queue, so a
                            # stalled normalization chain can't delay them
                            pre[b + 1] = emit_preamble(b + 1)
                        kT, v_nat, qTps = pre[b]
                        for hp in range(2):
                            qTp = qTps[hp]

                            # pending av pair, carried across q-block (jj)
                            # boundaries so the PE stream never drains:
                            # (av_tile, c0, e_tile, is_last, accT, jj)
                            prev = [None]
                            # two-deep deferred normalization chain.  Stage 1
                            # (post): avr copy + denominator reduce + recip.
                            # Stage 2 (post2), one group later: the
                            # partition-broadcast matmul + final scale + DMA,
                            # so the broadcast matmul never waits on the DVE
                            # recip at the PE queue head.  No gpsimd ops at
                            # all: the gpsimd queue carries collective
                            # triggers + their completion-gated staging
                            # copies, which would stall anything behind them.
                            post = [None]
                            post2 = [None]

                            def emit_post2(b=b, hp=hp):
                                if post2[0] is None:
                                    return
                                avr, smb, pjj = post2[0]
                                rbp = ps_rb.tile([128, SQ_BLK], f32,
                                                 tag="rbp", name="rbp")
                                nc.tensor.matmul(rbp[:], ones_row[:], smb[:],
                                                 start=True, stop=True)
                                at = at_o.tile([128, SQ_BLK], bf16,
                                               tag="at", name="at")
                                nc.vector.tensor_mul(at[:], avr[:], rbp[:])
                                nc.sync.dma_start(
                                    a2a_in[b][pjj,
                                              hp * 256:(hp + 1) * 256, :]
                                    .rearrange("(hh p) t -> p hh t", p=128),
                                    at[:].rearrange("p (hh t) -> p hh t",
                                                    hh=2))
                                post2[0] = None

                            def emit_post(emit_post2=emit_post2):
                                if post[0] is None:
                                    return
                                pav, pacc, pjj = post[0]
                                # the avr copy releases the av PSUM bank
                                avr = at_o.tile([128, SQ_BLK], bf16,
                                                tag="avr", name="avr")
                                nc.vector.tensor_copy(avr[:], pav[:])
                                sm = ps_sm.tile([1, SQ_BLK], f32,
                                                tag="sm", name="sm")
                                nc.tensor.matmul(sm[:], ones_col[:],
                                                 pacc[:, 0, :],
                                                 start=True, stop=True)
                                rs = at_nrm.tile([1, SQ_BLK], f32,
                                                 tag="rs", name="rs")
                                nc.vector.reciprocal_approx_fast(
                                    out=rs[:], in_=sm[:])
                                smb = at_nrm.tile([1, SQ_BLK], bf16,
                                                  tag="smb", name="smb")
                                nc.vector.tensor_copy(smb[:], rs[:])
                                emit_post2()
                                post2[0] = (avr, smb, pjj)
                                post[0] = None

                            def emit_av(nxt=None, v_nat=v_nat):
                                if prev[0] is None:
                                    prev[0] = nxt
                                    return
                                pav, pc0, pe, plast, pacc, pjj = prev[0]
                                nc.tensor.matmul(
                                    pav[:], v_nat[:, pc0, :], pe[:, 0, :],
                                    start=(pc0 == 0), stop=False)
                                nc.tensor.matmul(
                                    pav[:], v_nat[:, pc0 + 1, :], pe[:, 1, :],
                                    start=False, stop=plast)
                                emit_post()
                                if plast:
                                    post[0] = (pav, pacc, pjj)
                                prev[0] = nxt

                            # descending jj: the deep-pipeline blocks come
                            # first (better PE ramp), the shallow ones drain
                            # quickly right before the collective fires
                            for jj in reversed(range(NJJ)):
                                G = (jj + 1 if mask_mode == "causal"
                                     else N_TCHUNK // 2)
                                # (gpsimd elementwise measured 2-3x slower
                                # than DVE -- keep the exp-tile chain on DVE)
                                ee = nc.vector
                                qs = qTp[:, jj, :, :].rearrange(
                                    "p hh t -> p (hh t)")
                                accT = at_acc.tile([128, 2, SQ_BLK], bf16,
                                                   tag="accT")
                                av = ps_av.tile([128, SQ_BLK], f32, tag="av")
                                for g in range(G):
                                    c0 = 2 * g
                                    sp = ps_s.tile([128, 2, SQ_BLK], f32,
                                                   tag="s")
                                    nc.tensor.matmul(
                                        sp[:, 0, :],
                                        kT[:, c0 * 128:(c0 + 1) * 128],
                                        qs, start=True, stop=True)
                                    nc.tensor.matmul(
                                        sp[:, 1, :],
                                        kT[:, (c0 + 1) * 128:(c0 + 2) * 128],
                                        qs, start=True, stop=True)
                                    sp_flat = sp[:].rearrange(
                                        "p a q -> p (a q)")
                                    if mask_mode == "general":
                                        mt = at_mt.tile([128, 2, 2, A2A_TOK],
                                                        f32, tag="mt")
                                        for hh in range(2):
                                            nc.sync.dma_start(
                                                mt[:, :, hh, :],
                                                maskT.ap()[
                                                    c0 * 128:(c0 + 2) * 128,
                                                    jj * A2A_TOK:
                                                    (jj + 1) * A2A_TOK]
                                                .rearrange("(c p) q -> p c q",
                                                           p=128))
                                        nc.vector.tensor_add(
                                            sp_flat, sp_flat,
                                            mt[:].rearrange(
                                                "p c hh q -> p (c hh q)"))
                                    # first group's exp writes the
                                    # accumulator directly (except G==1,
                                    # where accT[:, 0, :] must become the
                                    # fold -- use a separate e there)
                                    e = (accT if g == 0 and G > 1 else
                                         at_e.tile([128, 2, SQ_BLK], bf16,
                                                   tag="e"))
                                    e_flat = e[:].rearrange("p a q -> p (a q)")
                                    nc.scalar.activation(e_flat, sp_flat, Exp,
                                                         scale=SCALE)
                                    if mask_mode == "causal" and g == G - 1:
                                        ee.tensor_mul(
                                            e_flat, e_flat,
                                            cm[:].rearrange(
                                                "p a hh t -> p (a hh t)"))
                                    # flush the pending av pair BEFORE the
                                    # accumulator update: when g==1 the
                                    # pending e aliases accT and must be
                                    # consumed before accT += e
                                    emit_av((av, c0, e, g == G - 1,
                                             accT, jj))
                                    if g > 0:
                                        ee.tensor_add(accT[:], accT[:],
                                                      e[:])
                                # fold the accumulator halves right after the
                                # last add, so the deferred (single) sm
                                # matmul reads a long-ready input
                                if G > 1:
                                    nc.vector.tensor_add(accT[:, 0, :],
                                                         accT[:, 0, :],
                                                         accT[:, 1, :])
                                else:
                                    nc.vector.tensor_add(accT[:, 0, :],
                                                         e[:, 0, :],
                                                         e[:, 1, :])
                            emit_av()
                            emit_post()
                            emit_post2()
                        # one collective per batch: triggers are ~100+us
                        # apart, so a trigger never waits on the previous
                        # collective (which would block this queue)
                        nc.gpsimd.collective_compute(
                            "AllToAll", mybir.AluOpType.bypass,
                            replica_groups=[list(range(N_CORES))],
                            ins=[a2a_in[b].opt()],
                            outs=[a2a_out[b].opt()],
                        )
                        # batch-0 a_sb loads directly behind the staging
                        # copies on the gpsimd queue -- exactly their data
                        # dependency.  (Batch-1's are deferred to between the
                        # wo passes: 32 pending gated DMA issues starve the
                        # weight-load stream of issue resources.)
                        if b == 0:
                            emit_a_loads(b)


                # ------------- stage 3: wo projection (2x256 owned tokens) ----
                # The first SPLIT m-blocks run on batch-0 tokens only (the
                # first collective's output) to cover the second collective's
                # latency; the rest run full-width with a single weight load;
                # then the split blocks' batch-1 halves re-stream their
                # weights.  Keeps ~50us of collective runway while loading
                # only 1.4x the wo weights (DMA traffic contends directly
                # with PE operand reads -- fewer bytes = faster matmuls).
                SPLIT = 16
                with tc.tile_pool(name="wo_o", bufs=4) as wo_o, \
                     tc.tile_pool(name="wo_ps", bufs=4, space="PSUM") as wo_ps:

                    def emit_wo(m, bb, w_sb2):
                        # bb None = both halves (still as per-half 256-wide
                        # matmuls: a 512-wide read spanning two separately
                        # batch-loaded a_sb halves raced intermittently)
                        bbs = [0, 1] if bb is None else [bb]
                        cols = slice(bbs[0] * A2A_TOK,
                                     (bbs[-1] + 1) * A2A_TOK)
                        w = cols.stop - cols.start
                        ps = wo_ps.tile([128, SQ_BLK], f32, tag="ps",
                                        name=f"ps{m}_{bb}")
                        for kc in range(NKC):
                            for bi in bbs:
                                c2 = slice(bi * A2A_TOK, (bi + 1) * A2A_TOK)
                                nc.tensor.matmul(
                                    ps[:, (bi - bbs[0]) * A2A_TOK:
                                       (bi - bbs[0] + 1) * A2A_TOK],
                                    w_sb2[:, kc, :], a_sb[:, kc, c2],
                                    start=(kc == 0 and bi == bbs[0]),
                                    stop=(kc == NKC - 1 and bi == bbs[-1]))
                        o_sb = wo_o.tile([128, SQ_BLK], bf16, tag="o")
                        nc.vector.tensor_copy(o_sb[:, 0:w], ps[:, 0:w])
                        nc.sync.dma_start(
                            out_d.ap()[m * 128:(m + 1) * 128, cols],
                            o_sb[:, 0:w])

                    def load_w(m, bb):
                        # dep-free weight loads on the scalar queue: idle
                        # after attention's exps, and a dep-free DMA can
                        # never block the queue head
                        w_sb2 = wo_w.tile([128, NKC, 128], bf16,
                                          tag="w", bufs=8,
                                          name=f"w{m}_{bb}")
                        nc.scalar.dma_start(w_sb2[:], woT4.ap()[m])
                        return w_sb2

                    for m in range(SPLIT):
                        emit_wo(m, 0, w0s[m] if m < 5 else load_w(m, 0))
                    # batch-1 a_sb loads: deferred past the runway blocks so
                    # their collective-gated issues don't starve the weight
                    # stream of DMA issue resources
                    emit_a_loads(1)
                    for m in range(SPLIT, NKC):
                        emit_wo(m, None, load_w(m, None))
                    for m in range(SPLIT):
                        emit_wo(m, 1, load_w(m, 1))

    nc.compile()
    return nc


def _get_program(mask_mode):
    if mask_mode not in _PROGRAMS:
        _PROGRAMS[mask_mode] = _build_program(mask_mode)
    return _PROGRAMS[mask_mode]


def _classify_mask(m2):
    if not m2.any():
        return "none"
    causal_ref = np.triu(np.full((S, S), -1e9, dtype=np.float32), k=1)
    return "causal" if np.array_equal(m2, causal_ref) else "general"


def _prep_inputs(x, freqs_cos, freqs_sin, mask, wq, wk, wv, wo):
    """Host-side sharding / layout prep shared by kernel() and test.py."""
    m2 = np.asarray(mask, np.float32).reshape(S, S)
    mask_mode = _classify_mask(m2)

    xT = np.ascontiguousarray(
        np.asarray(x, np.float32).reshape(TOK, D).T).astype(BF16)
    woT = np.asarray(wo, np.float32).T          # [hd_in, D_out]
    # pre-tile wo for contiguous stationary-block DMAs:
    # woT4[m, p, k, mcol] = woT[k*128+p, m*128+mcol]
    woT4 = np.ascontiguousarray(
        woT.reshape(NKC, 128, NKC, 128).transpose(2, 1, 0, 3)).astype(BF16)

    fc = np.asarray(freqs_cos, np.float32)
    fs = np.asarray(freqs_sin, np.float32)
    cos2 = np.ascontiguousarray(fc.T[_I_OF_P, :]).astype(BF16)    # [128, S]
    sgn = np.where(_IS_ODD, 1.0, -1.0).astype(np.float32)[:, None]
    sin2 = np.ascontiguousarray(fs.T[_I_OF_P, :] * sgn).astype(BF16)

    def permute_heads(w):
        w4 = np.asarray(w, np.float32).reshape(-1, HEAD_DIM, D)
        return w4[:, PERM, :].reshape(-1, D)

    wq_p = permute_heads(wq)
    wk_p = permute_heads(wk)
    wv = np.asarray(wv, np.float32)

    in_maps = []
    for c in range(N_CORES):
        wqkvT = np.ascontiguousarray(np.concatenate(
            [wq_p[c * 512:(c + 1) * 512], wk_p[c * 128:(c + 1) * 128],
             wv[c * 128:(c + 1) * 128]], axis=0).T).astype(BF16)   # [D, 768]
        m = {"xT": xT, "wqkvT": wqkvT, "woT4": woT4, "cos2": cos2, "sin2": sin2}
        if mask_mode == "general":
            m["maskT"] = np.ascontiguousarray(m2.T)
        in_maps.append(m)
    return mask_mode, in_maps


def kernel(x, start_pos, freqs_cos, freqs_sin, mask, cache_k, cache_v,
           wq, wk, wv, wo):
    from concourse.bass_utils import run_bass_kernel_spmd

    assert int(start_pos) == 0, "kernel compiled for start_pos == 0"
    mask_mode, in_maps = _prep_inputs(x, freqs_cos, freqs_sin, mask,
                                      wq, wk, wv, wo)
    nc = _get_program(mask_mode)
    res = run_bass_kernel_spmd(nc, in_maps, list(range(N_CORES)))
    out = np.empty((TOK, D), dtype=np.float32)
    for c in range(N_CORES):
        blk = np.asarray(res.results[c]["out"]).astype(np.float32)  # [D, 512]
        for b in range(B):
            rows = slice(b * S + A2A_TOK * c, b * S + A2A_TOK * (c + 1))
            out[rows, :] = blk[:, b * A2A_TOK:(b + 1) * A2A_TOK].T
    return out.reshape(B, S, D)



# revision 62
# speedup vs baseline: 1.0162x; 1.0064x over previous
"""Trainium2 Bass kernel for nn_Attention (llama-style attention block, GQA, RoPE).

bf16 dataflow (rel-err gate 2e-2; this kernel lands ~1.05e-2):
  - All matmul operands bf16 (PSUM accumulation stays f32).
  - Attention packs TWO q-heads per 512-wide moving operand (2 x 256
    tokens); causality handled at 256-token granularity.
  - Softmax: bf16 DVE accumulator sums the exp tiles (halves folded by a
    DVE add), ONE ones-matmul per q-block reduces it across partitions;
    the reciprocal is partition-broadcast via a K=1 ones matmul into PSUM
    (no gpsimd op), and the attention output is normalized on the sender.
  - v_nat (tokens-on-partitions V) comes straight from DRAM via the DMA
    transpose xbar: no PE transposes, no PSUM bank, no scalar drains.
  - The attention PE stream is software-pipelined across q-block
    boundaries (pending av pair + two-stage deferred normalization chain)
    so the PE never waits on the DVE chain at block ends.

Queue/collective discipline (the hard-won part):
  - ONE AllToAll per batch.  A collective trigger blocks its issue queue
    until the PREVIOUS collective completes (NRT straight-line order), so
    back-to-back collectives freeze the gpsimd queue when this core runs
    ahead of its peers (launch skew is tens of us).  Per-batch collectives
    are ~100+us apart and never block.
  - A non-Shared AllToAll output gets completion-gated staging copies on
    the TRIGGER'S queue (gpsimd) at the call site: nothing latency-
    sensitive may be emitted on gpsimd between a collective call and the
    point where its data is consumed anyway.  Attention therefore uses NO
    gpsimd ops at all; payload writes go on sync, weight loads on scalar
    (dep-free DMAs can never block a queue head), and the collective-gated
    a_sb loads are the only gpsimd DMAs (batched 8-per-batch -- a large
    pile of pending gated DMAs starves global DMA issue resources).
  - Concurrent DMA traffic directly slows PE matmuls (measured 220 ->
    408 ns/MM under saturating DMA), so wo weights stream only ~1.5x
    (first SPLIT m-blocks run per-batch-half for collective runway, the
    rest run both halves off a single weight load).

Distribution (8 NeuronCores, Megatron-style tensor parallel over heads):
  - Each core: 4 Q heads + matching KV head (wq/wk/wv output-dim sharded).
    Attention per-core in transposed dataflow (head_dim on partitions).
  - Per-batch AllToAlls reshard the attention output token-parallel; each
    core runs wo for its 2x256-token block against the full wo.
"""

import sys

if "/opt/trn_rl_repo" not in sys.path:
    sys.path.insert(0, "/opt/trn_rl_repo")

import numpy as np
import ml_dtypes

BF16 = ml_dtypes.bfloat16

N_CORES = 8
B, S, D = 2, 2048, 4096
N_HEADS = 32
N_KV_HEADS = 8
HEAD_DIM = 128
H_PER_CORE = N_HEADS // N_CORES          # 4 q heads per core
TOK = B * S                              # 4096 flattened tokens
QKV_M = H_PER_CORE * HEAD_DIM + 2 * HEAD_DIM  # 768 projection rows per core
PROJ_TOK = 512                           # token block in the projection stage
SQ_BLK = 512                             # moving width in attention (2 heads x 256)
A2A_TOK = 256                            # tokens per rank per per-batch AllToAll
NJJ = S // A2A_TOK                       # 8 q-blocks of 256 per batch
N_TCHUNK = S // HEAD_DIM                 # 16 key chunks per batch
SCALE = 1.0 / float(np.sqrt(HEAD_DIM))
NKC = D // 128                           # 32 contraction chunks

# partition permutation for RoPE: pair (even, odd) lives 16 partitions apart
# inside a 32-partition quadrant, so the rotation is a single stream_shuffle.
_P = np.arange(128)
_I_OF_P = 16 * (_P // 32) + (_P % 32) % 16          # rope pair index 0..63
_IS_ODD = (_P % 32) >= 16
PERM = (2 * _I_OF_P + _IS_ODD.astype(np.int64)).astype(np.int64)  # orig row in head block
SHUF_MASK = [(i + 16) % 32 for i in range(32)]

_PROGRAMS = {}


def _build_program(mask_mode):
    """Build + compile the SPMD program. mask_mode in {'causal', 'none', 'general'}."""
    import concourse.bass as bass
    import concourse.mybir as mybir
    import concourse.tile as tile
    from concourse import bacc
    from concourse.masks import make_identity

    f32 = mybir.dt.float32
    bf16 = mybir.dt.bfloat16
    Exp = mybir.ActivationFunctionType.Exp

    nc = bacc.Bacc("TRN2", target_bir_lowering=False, debug=False,
                   num_devices=N_CORES)

    xT = nc.dram_tensor("xT", [D, TOK], bf16, kind="ExternalInput")
    wqkvT = nc.dram_tensor("wqkvT", [D, QKV_M], bf16, kind="ExternalInput")
    # wo pre-tiled on host: [m_chunk, p, k_chunk, m_col] so each stationary
    # column-block DMA reads contiguous lines
    woT4 = nc.dram_tensor("woT4", [NKC, 128, NKC, 128], bf16, kind="ExternalInput")
    cos2 = nc.dram_tensor("cos2", [128, S], bf16, kind="ExternalInput")
    sin2 = nc.dram_tensor("sin2", [128, S], bf16, kind="ExternalInput")
    if mask_mode == "general":
        # additive mask stored transposed: maskT[k_pos, q_pos]
        maskT = nc.dram_tensor("maskT", [S, S], f32, kind="ExternalInput")
    out_d = nc.dram_tensor("out", [D, SQ_BLK], bf16, kind="ExternalOutput")

    xT_t = xT.ap().rearrange("(k p) t -> p k t", p=128)      # [128, 32, TOK]
    wqkvT_t = wqkvT.ap().rearrange("(k p) m -> p k m", p=128)  # [128, 32, 768]

    with tile.TileContext(nc) as tc:
        # at_kt/at_kv/at_q are hoisted alongside the projection pools so the
        # attention preamble DMAs (kT, v transposes, q loads) prefetch during
        # the projection stage instead of serializing at the pool boundary.
        with tc.tile_pool(name="const", bufs=1) as const, \
             tc.tile_pool(name="dram", bufs=1, space="DRAM") as dram, \
             tc.tile_pool(name="at_kt", bufs=2) as at_kt, \
             tc.tile_pool(name="at_kv", bufs=2) as at_kv, \
             tc.tile_pool(name="at_q", bufs=4) as at_q:
            # per-core v (transposed layout), split per batch; q/k stay in
            # SBUF (written directly by the projection RoPE)
            vd = [dram.tile([128, S], bf16, name=f"vd{b_}") for b_ in range(B)]
            # ONE payload tile per batch (all 4 heads).  Two collectives, not
            # four: a collective trigger instruction blocks its issue queue
            # until the PREVIOUS collective completes (NRT straight-line
            # ordering), so back-to-back collectives freeze whatever queue
            # hosts the trigger whenever this core runs ahead of its peers.
            # With one collective per batch the triggers are ~100+us apart
            # and never block.  NOTE also: a non-Shared AllToAll output makes
            # the framework emit post-collective staging copies on the SYNC
            # dma queue at the call site, gated on collective completion --
            # any later sync-queue DMA waits behind them.  So every
            # latency-sensitive DMA emitted after the first collective
            # (payload writes, weight loads, a_sb loads) is issued from a
            # NON-sync engine queue instead.
            a2a_in = [dram.tile([N_CORES, 2 * 256, A2A_TOK], bf16,
                                name=f"a2a_in{b_}") for b_ in range(B)]
            a2a_out = [dram.tile([N_CORES, 2 * 256, A2A_TOK], bf16,
                                 name=f"a2a_out{b_}") for b_ in range(B)]

            ones_col = const.tile([128, 1], bf16)     # lhsT for column sums
            nc.vector.memset(ones_col[:], 1.0)
            ones_row = const.tile([1, 128], bf16)     # lhsT for partition bcast
            nc.vector.memset(ones_row[:], 1.0)
            if mask_mode == "causal":
                # multiplicative 0/1 mask for the diagonal chunk-group:
                # cm[p, a, hh, t] = (t - p >= 128*a), same for both packed
                # heads hh
                cm = const.tile([128, 2, 2, A2A_TOK], bf16, name="cm")
                nc.gpsimd.memset(cm[:], 1.0)
                for a in range(2):
                    for hh in range(2):
                        nc.gpsimd.affine_select(
                            out=cm[:, a, hh, :],
                            in_=cm[:, a, hh, :],
                            pattern=[[1, A2A_TOK]], base=-128 * a,
                            channel_multiplier=-1,
                            compare_op=mybir.AluOpType.is_ge, fill=0.0,
                        )

            # q and k skip the DRAM roundtrip entirely: the projection's
            # RoPE adds write these attention tiles directly (their layouts
            # are clean slices of the projection output), saving 10MB of
            # DMA in the PE-critical projection phase.  Only v round-trips
            # (it needs the DMA transpose xbar, which reads DRAM).
            kT_t = [at_kt.tile([128, S], bf16, tag="kT", name=f"kT{b_}")
                    for b_ in range(B)]
            qT_t = [[at_q.tile([128, NJJ, 2, A2A_TOK], bf16, tag="qTp",
                               name=f"qTp{b_}_{hp_}") for hp_ in range(2)]
                    for b_ in range(B)]

            def emit_preamble(b):
                """v_nat transpose-loads for one batch's attention (tokens
                on partitions, via the DMA transpose xbar -- no PE
                transposes, no PSUM bank, no scalar drains)."""
                v_nat = at_kv.tile([128, N_TCHUNK, 128], bf16,
                                   tag="vn", name=f"vn{b}")
                for i in range(N_TCHUNK):
                    nc.sync.dma_start_transpose(
                        v_nat[:, i, :], vd[b][:, i * 128:(i + 1) * 128])
                return kT_t[b], v_nat, qT_t[b]

            pre = {}

            # ---------------- stage 1: fused QKV projection + RoPE ----------------
            # k-outer / m-inner with 6 live PSUM accumulation groups, so the
            # two half-K x tiles (xA, xB) double-buffer against each other.
            n_blk = TOK // PROJ_TOK   # 8
            HK = NKC // 2
            with tc.tile_pool(name="pj_w", bufs=1) as pj_w, \
                 tc.tile_pool(name="pj_x", bufs=3) as pj_x, \
                 tc.tile_pool(name="pj_cs", bufs=2) as pj_cs, \
                 tc.tile_pool(name="pj_t", bufs=2) as pj_t, \
                 tc.tile_pool(name="pj_o", bufs=2) as pj_o, \
                 tc.tile_pool(name="pj_ps", bufs=8, space="PSUM") as pj_ps:
                w_sb = pj_w.tile([128, NKC, QKV_M], bf16)
                x0 = slice(0, PROJ_TOK)
                xA0 = pj_x.tile([128, HK, PROJ_TOK], bf16, tag="xA")
                xB0 = pj_x.tile([128, HK, PROJ_TOK], bf16, tag="xB", bufs=2)
                # n=0 runs k-outer (below), so interleave x and w loads in
                # matching k-major pieces: the first matmul needs only
                # w[:, 0:2, :] + xA0[:, 0:2, :] (~0.6MB), not the full 10MB
                for kw in range(0, HK, 2):
                    nc.sync.dma_start(xA0[:, kw:kw + 2, :],
                                      xT_t[:, kw:kw + 2, x0])
                    nc.sync.dma_start(w_sb[:, kw:kw + 2, :],
                                      wqkvT_t[:, kw:kw + 2, :])
                for kw in range(HK, NKC, 2):
                    nc.sync.dma_start(xB0[:, kw - HK:kw - HK + 2, :],
                                      xT_t[:, kw:kw + 2, x0])
                    nc.sync.dma_start(w_sb[:, kw:kw + 2, :],
                                      wqkvT_t[:, kw:kw + 2, :])
                for n in range(n_blk):
                    s0 = (n * PROJ_TOK) % S  # position within the batch
                    bn = n // (S // PROJ_TOK)  # batch of this token block
                    cols = slice(n * PROJ_TOK, (n + 1) * PROJ_TOK)
                    bcols = slice(s0, s0 + PROJ_TOK)
                    if n == 0:
                        xA, xB = xA0, xB0
                    else:
                        xA = pj_x.tile([128, HK, PROJ_TOK], bf16, tag="xA")
                        xB = pj_x.tile([128, HK, PROJ_TOK], bf16, tag="xB", bufs=2)
                        nc.sync.dma_start(xA[:], xT_t[:, 0:HK, cols])
                        nc.sync.dma_start(xB[:], xT_t[:, HK:NKC, cols])
                    c_sb = pj_cs.tile([128, PROJ_TOK], bf16, tag="c")
                    s_sb = pj_cs.tile([128, PROJ_TOK], bf16, tag="s")
                    nc.sync.dma_start(c_sb[:], cos2.ap()[:, s0:s0 + PROJ_TOK])
                    nc.sync.dma_start(s_sb[:], sin2.ap()[:, s0:s0 + PROJ_TOK])
                    # n=0: k-outer / m-inner so the first matmul only waits on
                    # one small k-major x/w piece (fast startup ramp).
                    # n>=1: m-outer / k-inner so each head's full contraction
                    # finishes first and its RoPE drain overlaps the next
                    # head's matmuls (keeps the block tail short).
                    if n == 0:
                        pss = [pj_ps.tile([128, PROJ_TOK], f32, tag="ps",
                                          name=f"ps_{n}_{m_}")
                               for m_ in range(QKV_M // 128)]
                        for k in range(NKC):
                            xsb = xA if k < HK else xB
                            xi = k if k < HK else k - HK
                            for m_ in range(QKV_M // 128):
                                nc.tensor.matmul(
                                    pss[m_][:],
                                    w_sb[:, k, m_ * 128:(m_ + 1) * 128],
                                    xsb[:, xi, :],
                                    start=(k == 0), stop=(k == NKC - 1))
                    for m in range(QKV_M // 128):  # q0..q3, k, v
                        if n == 0:
                            ps = pss[m]
                        else:
                            ps = pj_ps.tile([128, PROJ_TOK], f32, tag="ps",
                                            name=f"ps_{n}_{m}")
                            for k in range(NKC):
                                xsb = xA if k < HK else xB
                                xi = k if k < HK else k - HK
                                nc.tensor.matmul(
                                    ps[:], w_sb[:, k, m * 128:(m + 1) * 128],
                                    xsb[:, xi, :],
                                    start=(k == 0), stop=(k == NKC - 1))
                        if m < 5:  # rope for q heads + k
                            # the final RoPE add writes the attention SBUF
                            # tile directly -- no DRAM roundtrip for q/k
                            if m < 4:
                                jj0 = s0 // A2A_TOK
                                dst_sb = (qT_t[bn][m // 2]
                                          [:, jj0:jj0 + 2, m % 2, :])
                                view = lambda a: a.rearrange(
                                    "p (jj t) -> p jj t", jj=2)
                            else:
                                dst_sb = kT_t[bn][:, s0:s0 + PROJ_TOK]
                                view = lambda a: a
                            tmp = pj_t.tile([128, PROJ_TOK], bf16, tag="tmp")
                            rot = pj_t.tile([128, PROJ_TOK], bf16, tag="rot")
                            t1 = pj_t.tile([128, PROJ_TOK], bf16, tag="t1")
                            nc.scalar.copy(tmp[:], ps[:])
                            nc.vector.stream_shuffle(rot[:], tmp[:], SHUF_MASK)
                            nc.vector.tensor_mul(t1[:], tmp[:], c_sb[:])
                            nc.vector.tensor_mul(rot[:], rot[:], s_sb[:])
                            nc.vector.tensor_add(dst_sb, view(t1[:]),
                                                 view(rot[:]))
                        else:
                            o_sb = pj_o.tile([128, PROJ_TOK], bf16, tag="o")
                            nc.scalar.copy(o_sb[:], ps[:])
                            nc.sync.dma_start(vd[bn][:, bcols], o_sb[:])
                    if n == S // PROJ_TOK - 1:
                        # emit batch-0 attention preamble DMAs here so they
                        # queue ahead of the remaining projection stores and
                        # run as soon as batch 0's q/k/v land in DRAM
                        pre[0] = emit_preamble(0)

            # ---------------- stage 2: attention + per-batch AllToAll --------
            # Two q-heads packed side by side in the 512-wide moving operand
            # (2 x 256 tokens); causality handled at 256-token granularity.
            # wo_w is opened before the attention pools so the wo weight
            # prefetch streams during attention / the collectives. The batch-0
            # collective overlaps batch-1 attention; Pool-stream ordering is
            # kept safe by deferring each hp's normalization broadcast to its
            # end (so it is never queued behind a collective it doesn't need).
            with tc.tile_pool(name="wo_w", bufs=5) as wo_w:
                a_sb = wo_w.tile([128, NKC, SQ_BLK], bf16, tag="a", bufs=1)

                def emit_a_loads(bb):
                    # issued from the gpsimd queue (the sync queue would
                    # stall payload writes behind these collective-gated
                    # loads; the scalar queue lets the tile scheduler
                    # interleave them ahead of batch-1's exps).  Batched to 8
                    # issues: a large pile of pending gated DMA instructions
                    # starves the machine of global DMA issue resources when
                    # the collective is slow (high peer skew).
                    for r_ in range(N_CORES):
                        nc.gpsimd.dma_start(
                            a_sb[:, 4 * r_:4 * r_ + 4,
                                 bb * A2A_TOK:(bb + 1) * A2A_TOK],
                            a2a_out[bb][r_].rearrange("(hh p) t -> p hh t",
                                                      p=128))

                # first wo weight tiles prefetch on the sync queue, emitted
                # BEFORE the first collective call site (after it, the sync
                # queue is blocked by collective staging copies)
                w0s = []
                for m_ in range(8):
                    w0 = wo_w.tile([128, NKC, 128], bf16,
                                   tag="w", bufs=8, name=f"w{m_}_0")
                    nc.sync.dma_start(w0[:], woT4.ap()[m_])
                    w0s.append(w0)
                with tc.tile_pool(name="at_e", bufs=7) as at_e, \
                     tc.tile_pool(name="at_acc", bufs=3) as at_acc, \
                     tc.tile_pool(name="at_nrm", bufs=4) as at_nrm, \
                     tc.tile_pool(name="at_o", bufs=3) as at_o, \
                     tc.tile_pool(name="at_mt", bufs=4) as at_mt, \
                     tc.tile_pool(name="ps_s", bufs=2, space="PSUM") as ps_s, \
                     tc.tile_pool(name="ps_av", bufs=2, space="PSUM") as ps_av, \
                     tc.tile_pool(name="ps_sm", bufs=1, space="PSUM") as ps_sm, \
                     tc.tile_pool(name="ps_rb", bufs=1, space="PSUM") as ps_rb:
                    for b in range(B):
                        if b not in pre:
                            pre[b] = emit_preamble(b)
                        if b + 1 < B and b + 1 not in pre:
                            # emit the next batch's preamble loads now, ahead
                            # of this batch's at-dmas in the DMA queue, so a
                            # stalled normalization chain can't delay them
                            pre[b + 1] = emit_preamble(b + 1)
                        kT, v_nat, qTps = pre[b]
                        for hp in range(2):
                            qTp = qTps[hp]

                            # pending av pair, carried across q-block (jj)
                            # boundaries so the PE stream never drains:
                            # (av_tile, c0, e_tile, is_last, accT, jj)
                            prev = [None]
                            # two-deep deferred normalization chain.  Stage 1
                            # (post): avr copy + denominator reduce + recip.
                            # Stage 2 (post2), one group later: the
                            # partition-broadcast matmul + final scale + DMA,
                            # so the broadcast matmul never waits on the DVE
                            # recip at the PE queue head.  No gpsimd ops at
                            # all: the gpsimd queue carries collective
                            # triggers + their completion-gated staging
                            # copies, which would stall anything behind them.
                            post = [None]
                            post2 = [None]

                            def emit_post2(b=b, hp=hp):
                                if post2[0] is None:
                                    return
                                avr, smb, pjj = post2[0]
                                rbp = ps_rb.tile([128, SQ_BLK], f32,
                                                 tag="rbp", name="rbp")
                                nc.tensor.matmul(rbp[:], ones_row[:], smb[:],
                                                 start=True, stop=True)
                                at = at_o.tile([128, SQ_BLK], bf16,
                                               tag="at", name="at")
                                nc.vector.tensor_mul(at[:], avr[:], rbp[:])
                                nc.sync.dma_start(
                                    a2a_in[b][pjj,
                                              hp * 256:(hp + 1) * 256, :]
                                    .rearrange("(hh p) t -> p hh t", p=128),
                                    at[:].rearrange("p (hh t) -> p hh t",
                                                    hh=2))
                                post2[0] = None

                            def emit_post(emit_post2=emit_post2):
                                if post[0] is None:
                                    return
                                pav, pacc, pjj = post[0]
                                # the avr copy releases the av PSUM bank
                                avr = at_o.tile([128, SQ_BLK], bf16,
                                                tag="avr", name="avr")
                                nc.vector.tensor_copy(avr[:], pav[:])
                                sm = ps_sm.tile([1, SQ_BLK], f32,
                                                tag="sm", name="sm")
                                nc.tensor.matmul(sm[:], ones_col[:],
                                                 pacc[:, 0, :],
                                                 start=True, stop=True)
                                rs = at_nrm.tile([1, SQ_BLK], f32,
                                                 tag="rs", name="rs")
                                nc.vector.reciprocal_approx_fast(
                                    out=rs[:], in_=sm[:])
                                smb = at_nrm.tile([1, SQ_BLK], bf16,
                                                  tag="smb", name="smb")
                                nc.vector.tensor_copy(smb[:], rs[:])
                                emit_post2()
                                post2[0] = (avr, smb, pjj)
                                post[0] = None

                            def emit_av(nxt=None, v_nat=v_nat):
                                if prev[0] is None:
                                    prev[0] = nxt
                                    return
                                pav, pc0, pe, plast, pacc, pjj = prev[0]
                                nc.tensor.matmul(
                                    pav[:], v_nat[:, pc0, :], pe[:, 0, :],
                                    start=(pc0 == 0), stop=False)
                                nc.tensor.matmul(
                                    pav[:], v_nat[:, pc0 + 1, :], pe[:, 1, :],
                                    start=False, stop=plast)
                                emit_post()
                                if plast:
                                    post[0] = (pav, pacc, pjj)
                                prev[0] = nxt

                            # descending jj: the deep-pipeline blocks come
                            # first (better PE ramp), the shallow ones drain
                            # quickly right before the collective fires
                            for jj in reversed(range(NJJ)):
                                G = (jj + 1 if mask_mode == "causal"
                                     else N_TCHUNK // 2)
                                # (gpsimd elementwise measured 2-3x slower
                                # than DVE -- keep the exp-tile chain on DVE)
                                ee = nc.vector
                                qs = qTp[:, jj, :, :].rearrange(
                                    "p hh t -> p (hh t)")
                                accT = at_acc.tile([128, 2, SQ_BLK], bf16,
                                                   tag="accT")
                                av = ps_av.tile([128, SQ_BLK], f32, tag="av")
                                for g in range(G):
                                    c0 = 2 * g
                                    sp = ps_s.tile([128, 2, SQ_BLK], f32,
                                                   tag="s")
                                    nc.tensor.matmul(
                                        sp[:, 0, :],
                                        kT[:, c0 * 128:(c0 + 1) * 128],
                                        qs, start=True, stop=True)
                                    nc.tensor.matmul(
                                        sp[:, 1, :],
                                        kT[:, (c0 + 1) * 128:(c0 + 2) * 128],
                                        qs, start=True, stop=True)
                                    sp_flat = sp[:].rearrange(
                                        "p a q -> p (a q)")
                                    if mask_mode == "general":
                                        mt = at_mt.tile([128, 2, 2, A2A_TOK],
                                                        f32, tag="mt")
                                        for hh in range(2):
                                            nc.sync.dma_start(
                                                mt[:, :, hh, :],
                                                maskT.ap()[
                                                    c0 * 128:(c0 + 2) * 128,
                                                    jj * A2A_TOK:
                                                    (jj + 1) * A2A_TOK]
                                                .rearrange("(c p) q -> p c q",
                                                           p=128))
                                        nc.vector.tensor_add(
                                            sp_flat, sp_flat,
                                            mt[:].rearrange(
                                                "p c hh q -> p (c hh q)"))
                                    # first group's exp writes the
                                    # accumulator directly (except G==1,
                                    # where accT[:, 0, :] must become the
                                    # fold -- use a separate e there)
                                    e = (accT if g == 0 and G > 1 else
                                         at_e.tile([128, 2, SQ_BLK], bf16,
                                                   tag="e"))
                                    e_flat = e[:].rearrange("p a q -> p (a q)")
                                    nc.scalar.activation(e_flat, sp_flat, Exp,
                                                         scale=SCALE)
                                    if mask_mode == "causal" and g == G - 1:
                                        ee.tensor_mul(
                                            e_flat, e_flat,
                                            cm[:].rearrange(
                                                "p a hh t -> p (a hh t)"))
                                    # flush the pending av pair BEFORE the
                                    # accumulator update: when g==1 the
                                    # pending e aliases accT and must be
                                    # consumed before accT += e
                                    emit_av((av, c0, e, g == G - 1,
                                             accT, jj))
                                    if g > 0:
                                        ee.tensor_add(accT[:], accT[:],
                                                      e[:])
                                # fold the accumulator halves right after the
                                # last add, so the deferred (single) sm
                                # matmul reads a long-ready input
                                if G > 1:
                                    nc.vector.tensor_add(accT[:, 0, :],
                                                         accT[:, 0, :],
                                                         accT[:, 1, :])
                                else:
                                    nc.vector.tensor_add(accT[:, 0, :],
                                                         e[:, 0, :],
                                                         e[:, 1, :])
                            emit_av()
                            emit_post()
                            emit_post2()
                        # one collective per batch: triggers are ~100+us
                        # apart, so a trigger never waits on the previous
                        # collective (which would block this queue)
                        nc.gpsimd.collective_compute(
                            "AllToAll", mybir.AluOpType.bypass,
                            replica_groups=[list(range(N_CORES))],
                            ins=[a2a_in[b].opt()],
                            outs=[a2a_out[b].opt()],
                        )
                        # batch-0 a_sb loads directly behind the staging
                        # copies on the gpsimd queue -- exactly their data
                        # dependency.  (Batch-1's are deferred to between the
                        # wo passes: 32 pending gated DMA issues starve the
                        # weight-load stream of issue resources.)
                        if b == 0:
                            emit_a_loads(b)


                # ------------- stage 3: wo projection (2x256 owned tokens) ----
                # The first SPLIT m-blocks run on batch-0 tokens only (the
                # first collective's output) to cover the second collective's
                # latency; the rest run full-width with a single weight load;
                # then the split blocks' batch-1 halves re-stream their
                # weights.  Keeps ~50us of collective runway while loading
                # only 1.4x the wo weights (DMA traffic contends directly
                # with PE operand reads -- fewer bytes = faster matmuls).
                SPLIT = 16
                with tc.tile_pool(name="wo_o", bufs=4) as wo_o, \
                     tc.tile_pool(name="wo_ps", bufs=4, space="PSUM") as wo_ps:

                    def emit_wo(m, bb, w_sb2):
                        # bb None = both halves (still as per-half 256-wide
                        # matmuls: a 512-wide read spanning two separately
                        # batch-loaded a_sb halves raced intermittently)
                        bbs = [0, 1] if bb is None else [bb]
                        cols = slice(bbs[0] * A2A_TOK,
                                     (bbs[-1] + 1) * A2A_TOK)
                        w = cols.stop - cols.start
                        ps = wo_ps.tile([128, SQ_BLK], f32, tag="ps",
                                        name=f"ps{m}_{bb}")
                        for kc in range(NKC):
                            for bi in bbs:
                                c2 = slice(bi * A2A_TOK, (bi + 1) * A2A_TOK)
                                nc.tensor.matmul(
                                    ps[:, (bi - bbs[0]) * A2A_TOK:
                                       (bi - bbs[0] + 1) * A2A_TOK],
                                    w_sb2[:, kc, :], a_sb[:, kc, c2],
                                    start=(kc == 0 and bi == bbs[0]),
                                    stop=(kc == NKC - 1 and bi == bbs[-1]))
                        o_sb = wo_o.tile([128, SQ_BLK], bf16, tag="o")
                        nc.vector.tensor_copy(o_sb[:, 0:w], ps[:, 0:w])
                        nc.sync.dma_start(
                            out_d.ap()[m * 128:(m + 1) * 128, cols],
                            o_sb[:, 0:w])

                    def load_w(m, bb):
                        # dep-free weight loads on the scalar queue: idle
                        # after attention's exps, and a dep-free DMA can
                        # never block the queue head
                        w_sb2 = wo_w.tile([128, NKC, 128], bf16,
                                          tag="w", bufs=8,
                                          name=f"w{m}_{bb}")
                        nc.scalar.dma_start(w_sb2[:], woT4.ap()[m])
                        return w_sb2

                    for m in range(SPLIT):
                        emit_wo(m, 0, w0s[m] if m < 5 else load_w(m, 0))
                    # batch-1 a_sb loads: deferred past the runway blocks so
                    # their collective-gated issues don't starve the weight
                    # stream of DMA issue resources
                    emit_a_loads(1)
                    for m in range(SPLIT, NKC):
                        emit_wo(m, None, load_w(m, None))
                    for m in range(SPLIT):
                        emit_wo(m, 1, load_w(m, 1))

    nc.compile()
    return nc


def _get_program(mask_mode):
    if mask_mode not in _PROGRAMS:
        _PROGRAMS[mask_mode] = _build_program(mask_mode)
    return _PROGRAMS[mask_mode]


def _classify_mask(m2):
    if not m2.any():
        return "none"
    causal_ref = np.triu(np.full((S, S), -1e9, dtype=np.float32), k=1)
    return "causal" if np.array_equal(m2, causal_ref) else "general"


def _prep_inputs(x, freqs_cos, freqs_sin, mask, wq, wk, wv, wo):
    """Host-side sharding / layout prep shared by kernel() and test.py."""
    m2 = np.asarray(mask, np.float32).reshape(S, S)
    mask_mode = _classify_mask(m2)

    xT = np.ascontiguousarray(
        np.asarray(x, np.float32).reshape(TOK, D).T).astype(BF16)
    woT = np.asarray(wo, np.float32).T          # [hd_in, D_out]
    # pre-tile wo for contiguous stationary-block DMAs:
    # woT4[m, p, k, mcol] = woT[k*128+p, m*128+mcol]
    woT4 = np.ascontiguousarray(
        woT.reshape(NKC, 128, NKC, 128).transpose(2, 1, 0, 3)).astype(BF16)

    fc = np.asarray(freqs_cos, np.float32)
    fs = np.asarray(freqs_sin, np.float32)
    cos2 = np.ascontiguousarray(fc.T[_I_OF_P, :]).astype(BF16)    # [128, S]
    sgn = np.where(_IS_ODD, 1.0, -1.0).astype(np.float32)[:, None]
    sin2 = np.ascontiguousarray(fs.T[_I_OF_P, :] * sgn).astype(BF16)

    def permute_heads(w):
        w4 = np.asarray(w, np.float32).reshape(-1, HEAD_DIM, D)
        return w4[:, PERM, :].reshape(-1, D)

    wq_p = permute_heads(wq)
    wk_p = permute_heads(wk)
    wv = np.asarray(wv, np.float32)

    in_maps = []
    for c in range(N_CORES):
        wqkvT = np.ascontiguousarray(np.concatenate(
            [wq_p[c * 512:(c + 1) * 512], wk_p[c * 128:(c + 1) * 128],
             wv[c * 128:(c + 1) * 128]], axis=0).T).astype(BF16)   # [D, 768]
        m = {"xT": xT, "wqkvT": wqkvT, "woT4": woT4, "cos2": cos2, "sin2": sin2}
        if mask_mode == "general":
            m["maskT"] = np.ascontiguousarray(m2.T)
        in_maps.append(m)
    return mask_mode, in_maps


def kernel(x, start_pos, freqs_cos, freqs_sin, mask, cache_k, cache_v,
           wq, wk, wv, wo):
    from concourse.bass_utils import run_bass_kernel_spmd

    assert int(start_pos) == 0, "kernel compiled for start_pos == 0"
    mask_mode, in_maps = _prep_inputs(x, freqs_cos, freqs_sin, mask,
                                      wq, wk, wv, wo)
    nc = _get_program(mask_mode)
    res = run_bass_kernel_spmd(nc, in_maps, list(range(N_CORES)))
    out = np.empty((TOK, D), dtype=np.float32)
    for c in range(N_CORES):
        blk = np.asarray(res.results[c]["out"]).astype(np.float32)  # [D, 512]
        for b in range(B):
            rows = slice(b * S + A2A_TOK * c, b * S + A2A_TOK * (c + 1))
            out[rows, :] = blk[:, b * A2A_TOK:(b + 1) * A2A_TOK].T
    return out.reshape(B, S, D)

